# revision 28
# baseline (speedup 1.0000x reference)
"""Leaky-integrator linear recurrence kernel for Trainium2.

u_t = TAU * u_{t-1} + x_t along the last (time) axis of x[32, 1024, 2048] f32.

Strategy: data-parallel across 8 NeuronCores (4 batches each), 16-bit HBM
traffic (the 2e-2 tolerance dwarfs the quantization error), and a HYBRID
compute split that keeps every engine under the DMA stream time:

* 2048 rows/core via the Tensor engine as a *banded matmul* in a
  host-transposed layout: since TAU^129 < 2e-6, u_t is (to float
  precision) a windowed sum over the last 256 steps, computed per 128-step
  time-block as two accumulating 128x128-stationary matmuls (cross-block
  band A + triangular band B; block 0 skips A). PSUM f32 -> SBUF bf16
  downcasts run on the Scalar engine.
* 2048 rows/core via the Vector engine's hardware scan
  (TensorTensorScanArith, fp32 internal state), fp16 end-to-end on this
  path (fp16(0.9)=0.89990; bf16's 0.8984 compounds over the recurrence).
  16 scans of [128, 2048] at ~4.3 us each, emitted early so the Vector
  engine runs continuously, with each scan's output DMA'd immediately
  (whole-super-tile DMAs starve the write stream in 1.5 MB bursts).

DMA layout rules learned on this part: every large DMA must be
128-partition aligned (partial-partition APs defeat balance_dma_aps and
serialize the whole transfer onto one DMA engine), and per-partition HBM
lines must be a multiple of the 4 KiB SDMA packet (runt packets cost
~10-15% of line rate). Hence the matmul path's tensors are host-packed in
block PAIRS — xp row q*128+p holds time-rows 256q+p and 256q+128+p side
by side — giving 8 KiB lines, and the scan path packs 4 consecutive rows
per partition line (16 KiB input runs, 4 KiB per-scan output runs).

Engine assignment: Sync issues input DMAs, Scalar issues output DMAs (two
HWDGE rings, so input prefetch never head-of-line blocks behind output
drain). _dedup_ldweights() drops redundant PE weight reloads
(tile_legalize splits each matmul into InstLdweights + a non-self-loading
InstMatmult; consecutive identical loads are dead weight, ~100 ns each).

The walrus build in this container allows at most ONE embedded sync-wait
per engine instruction (two on EventSemaphore); Tile's wait assignment can
attach several. _split_excess_waits() hoists the extras onto standalone
EventSemaphore instructions inserted immediately before, on the same
engine — conservative but correct, since every awaited semaphore's
producer precedes the waiter in the scheduled program order.
"""

import numpy as np
import ml_dtypes

import concourse.bass as bass
import concourse.mybir as mybir
from concourse.bass_utils import run_bass_kernel_spmd
from concourse.tile import TileContext

TAU = 0.9
B, F, T = 32, 1024, 2048
N_CORES = 8
B_PER_CORE = B // N_CORES          # 4
ROWS = B_PER_CORE * F              # 4096 independent recurrences per core
P = 128
N_BLK = T // P                     # 16 time-blocks
CHUNK = 512                        # PSUM bank width (f32)

MM_ROWS = 2048                     # rows on the TensorE matmul path
SC_ROWS = ROWS - MM_ROWS           # 2048 rows on the VectorE scan path
N_CHUNK = MM_ROWS // CHUNK         # 4
R_PER_P = 4                        # scan rows packed per partition line
N_SSUP = SC_ROWS // (P * R_PER_P)  # 4 scan super-tiles [128, 4*T]
N_PAIR = N_BLK // 2                # 8 slab pairs on the matmul path

NP_DT = ml_dtypes.bfloat16
MYBIR_DT = mybir.dt.bfloat16
SC_NP_DT = np.float16
SC_MYBIR_DT = mybir.dt.float16

_nc_cache = None
_coef_cache = None
last_results = None  # BassKernelResults from the most recent run (for test.py)


def _split_excess_waits(nc: bass.Bass) -> None:
    for fn in nc.m.functions:
        for blk in fn.blocks:
            out = []
            changed = False
            for inst in blk.instructions:
                si = inst.sync_info
                waits = list(si.on_wait) if si is not None else []
                cap = 2 if inst.opcode == "EventSemaphore" else 1
                if len(waits) <= cap:
                    out.append(inst)
                    continue
                changed = True
                # On DMAs keep a queue-ordering (DMAHW*) wait embedded so
                # queue-level throttling stays at the queue; otherwise keep
                # the last wait.
                keep_idx = len(waits) - 1
                if inst.opcode == "DMACopy":
                    for k, w in enumerate(waits):
                        if (w.ant_name or "").startswith("DMA"):
                            keep_idx = k
                            break
                rest = [w for j, w in enumerate(waits) if j != keep_idx]
                for j in range(0, len(rest), 2):
                    out.append(
                        mybir.InstEventSemaphore(
                            name=f"{inst.name}-xw{j}",
                            opcode="EventSemaphore",
                            engine=inst.engine,
                            debug=inst.debug,
                            sync_info=mybir.SyncInfo(
                                on_wait=rest[j : j + 2], on_update=[]
                            ),
                        )
                    )
                inst.sync_info = mybir.SyncInfo(
                    on_wait=[waits[keep_idx]], on_update=list(si.on_update)
                )
                out.append(inst)
            if changed:
                blk.instructions = out


def _dedup_ldweights(nc: bass.Bass) -> None:
    """Drop PE weight reloads that reload the already-loaded stationary."""
    for fn in nc.m.functions:
        for blk in fn.blocks:
            out = []
            changed = False
            last_sig = None
            for inst in blk.instructions:
                if inst.opcode == "Matmult":
                    out.append(inst)
                    continue
                if inst.opcode != "Ldweights":
                    if inst.engine == mybir.EngineType.PE and inst.opcode not in (
                        "EventSemaphore",
                    ):
                        last_sig = None
                    out.append(inst)
                    continue
                a = inst.ins[0]
                sig = (a.memref, a.offset, str(a.ap), str(a.dtype))
                if sig != last_sig:
                    last_sig = sig
                    out.append(inst)
                    continue
                changed = True
                si = inst.sync_info
                waits = list(si.on_wait) if si is not None else []
                upds = list(si.on_update) if si is not None else []
                if waits or upds:
                    for j in range(0, max(len(waits), 1), 2):
                        out.append(
                            mybir.InstEventSemaphore(
                                name=f"{inst.name}-lw{j}",
                                opcode="EventSemaphore",
                                engine=inst.engine,
                                debug=inst.debug,
                                sync_info=mybir.SyncInfo(
                                    on_wait=waits[j : j + 2],
                                    on_update=upds if j == 0 else [],
                                ),
                            )
                        )
            if changed:
                blk.instructions = out


def _coef() -> np.ndarray:
    # [P, 2P] = [A | B] packed side by side (one SBUF tile, one DMA):
    #   A[k, m] = TAU^(m+128-k)                (cross-block band)
    #   B[k, m] = TAU^(m-k) for k <= m else 0  (triangular band)
    k = np.arange(2 * P)[:, None]
    m = np.arange(P)[None, :]
    e = m + P - k
    c = np.where(e >= 0, TAU ** np.maximum(e, 0).astype(np.float64), 0.0)
    return np.ascontiguousarray(np.hstack([c[:P], c[P:]]).astype(NP_DT))


def _build() -> bass.Bass:
    nc = bass.Bass()
    xp = nc.dram_tensor("xp", [N_PAIR * P, 2 * MM_ROWS], MYBIR_DT, kind="ExternalInput")
    xs = nc.dram_tensor("xs", [SC_ROWS, T], SC_MYBIR_DT, kind="ExternalInput")
    coef = nc.dram_tensor("coef", [P, 2 * P], MYBIR_DT, kind="ExternalInput")
    yp = nc.dram_tensor("yp", [N_PAIR * P, 2 * MM_ROWS], MYBIR_DT, kind="ExternalOutput")
    ys = nc.dram_tensor("ys", [SC_ROWS, T], SC_MYBIR_DT, kind="ExternalOutput")

    xp_r = xp.rearrange("(q p) w -> q p w", p=P)   # 8 pairs [128, 2*MM_ROWS]
    yp_r = yp.rearrange("(q p) w -> q p w", p=P)
    xs_r = xs.rearrange("(i p s) t -> i p (s t)", p=P, s=R_PER_P)
    ys_r = ys.rearrange("(i p s) t -> i p (s t)", p=P, s=R_PER_P)

    with TileContext(nc) as tc:
        with (
            tc.tile_pool(name="const", bufs=1) as cpool,
            tc.tile_pool(name="in", bufs=5) as ipool,
            tc.tile_pool(name="out", bufs=3) as opool,
            tc.tile_pool(name="sin", bufs=2) as sipool,
            tc.tile_pool(name="sout", bufs=2) as sopool,
            tc.tile_pool(name="psum", bufs=8, space="PSUM") as ppool,
        ):
            cf = cpool.tile([P, 2 * P], MYBIR_DT)
            nc.sync.dma_start(out=cf[:], in_=coef[:])
            cA = cf[:, 0:P]
            cB = cf[:, P : 2 * P]
            tau = cpool.tile([P, T], SC_MYBIR_DT)
            nc.vector.memset(tau[:], TAU)

            def scan_supertile(k):
                sin = sipool.tile([P, R_PER_P * T], SC_MYBIR_DT)
                nc.sync.dma_start(out=sin[:], in_=xs_r[k])
                sout = sopool.tile([P, R_PER_P * T], SC_MYBIR_DT)
                for r in range(R_PER_P):
                    sl = slice(r * T, (r + 1) * T)
                    nc.vector.tensor_tensor_scan(
                        sout[:, sl], tau[:], sin[:, sl], 0.0,
                        mybir.AluOpType.mult, mybir.AluOpType.add,
                    )
                    # stream each scan's output immediately: whole-tile DMAs
                    # gate on ~17 us of serial scans and starve the write
                    # stream in 1.5 MB bursts
                    nc.scalar.dma_start(out=ys_r[k][:, sl], in_=sout[:, sl])

            def slab(i):
                return pairs[i // 2][:, (i % 2) * MM_ROWS : (i % 2 + 1) * MM_ROWS]

            # Vector is the longest compute pole (~70 us of scans): start it
            # immediately and keep it fed; pair 0 queues right behind.
            scan_supertile(0)

            pairs = []
            for q in range(N_PAIR):
                sp = ipool.tile([P, 2 * MM_ROWS], MYBIR_DT)
                nc.sync.dma_start(out=sp[:], in_=xp_r[q])
                pairs.append(sp)

                upair = opool.tile([P, 2 * MM_ROWS], MYBIR_DT)
                for j in range(2):
                    i = 2 * q + j
                    order = list(range(N_CHUNK))
                    if i % 2:
                        order.reverse()
                    pts = {}
                    for c in order:
                        pt = ppool.tile([P, CHUNK], mybir.dt.float32)
                        pts[c] = pt
                        if i > 0:
                            nc.tensor.matmul(
                                pt[:], lhsT=cA[:],
                                rhs=slab(i - 1)[:, c * CHUNK : (c + 1) * CHUNK],
                                start=True, stop=False,
                            )
                    for c in order:
                        osl = slice(j * MM_ROWS + c * CHUNK, j * MM_ROWS + (c + 1) * CHUNK)
                        nc.tensor.matmul(
                            pts[c][:], lhsT=cB[:],
                            rhs=slab(i)[:, c * CHUNK : (c + 1) * CHUNK],
                            start=(i == 0), stop=True,
                        )
                        nc.scalar.copy(upair[:, osl], pts[c][:])
                        if q == N_PAIR - 1 and j == 1:
                            # stream the final block's output per chunk
                            nc.scalar.dma_start(out=yp_r[q][:, osl], in_=upair[:, osl])
                    if q == N_PAIR - 1 and j == 0:
                        nc.scalar.dma_start(
                            out=yp_r[q][:, 0:MM_ROWS], in_=upair[:, 0:MM_ROWS]
                        )
                if q != N_PAIR - 1:
                    nc.scalar.dma_start(out=yp_r[q], in_=upair[:])
                if q >= 1:
                    pairs[q - 1] = None

                # remaining scan super-tiles spread over the early pairs so
                # Vector never idles (each super-tile is ~17 us of scans)
                if q + 1 < N_SSUP:
                    scan_supertile(q + 1)

    _dedup_ldweights(nc)
    _split_excess_waits(nc)
    return nc


def kernel(x: np.ndarray, **_unused) -> np.ndarray:
    global _nc_cache, _coef_cache, last_results
    if _nc_cache is None:
        _nc_cache = _build()
        _coef_cache = _coef()
    nc = _nc_cache

    x = np.asarray(x)
    assert x.shape == (B, F, T), x.shape
    xr = x.reshape(N_CORES, ROWS, T)
    in_maps = []
    for c in range(N_CORES):
        # matmul path: transpose to [T, MM_ROWS] then interleave time-rows
        # t and t+128 of each 256-step pair side by side (8 KiB HBM lines)
        xt = xr[c, 0:MM_ROWS].astype(NP_DT).T          # [T, MM_ROWS]
        xpm = np.ascontiguousarray(
            xt.reshape(N_PAIR, 2, P, MM_ROWS)
            .transpose(0, 2, 1, 3)
            .reshape(N_PAIR * P, 2 * MM_ROWS)
        )
        xsm = np.ascontiguousarray(xr[c, MM_ROWS:], dtype=SC_NP_DT)
        in_maps.append({"xp": xpm, "xs": xsm, "coef": _coef_cache})
    last_results = run_bass_kernel_spmd(
        nc, in_maps, core_ids=list(range(N_CORES))
    )
    outs = []
    for r in last_results.results:
        yt = (
            r["yp"]
            .reshape(N_PAIR, P, 2, MM_ROWS)
            .transpose(0, 2, 1, 3)
            .reshape(T, MM_ROWS)
        )
        u = np.concatenate(
            [yt.T.astype(np.float32), r["ys"].astype(np.float32)], axis=0
        )
        outs.append(u.reshape(B_PER_CORE, F, T))
    return np.concatenate(outs, axis=0)
